# revision 66
# baseline (speedup 1.0000x reference)
"""GATv2 (2-layer, 8-head) message-passing kernel for 8 Trainium2 NeuronCores.

Sharding: nodes (and incoming edges) are partitioned across 8 cores by
destination; weights replicated.  Per core, dst nodes are LPT-bin-packed into
40 blocks of 32 nodes (balancing edge counts so every block needs the same
t_blk=8 tiles of 128 edges).  Layer-0 source features xl0 are computed
replicated directly from x via the host-folded weight [Wp;bp]@Wl0
(contraction dim 8), written to DRAM (512B rows), and gathered per edge with
SWDGE.  Layer-1 xl is projected per chunk of 256 own nodes as layer-0 blocks
retire and a chunked AllGather distributes it, overlapping the collective
with compute.

SWDGE descriptor generation has a ~4.4us per-ucode-activation overhead but
batches all pending gather instructions into one activation, so the gather
pool is kept deep (6 bufs x 2 halves) to keep many gathers pending; that
drops the effective cost from ~9ns/edge to ~3ns/edge.

Edge math per 128-edge tile: one batched ACT copy preloads the gathered xl
of a whole 4-tile q-group into PSUM, one PE matmul per tile accumulates the
indicator-expanded xr on top (feat = xl[src]+xr[dst]); ACT Lrelu, DVE mul by
att + per-head reduce, ACT exp, DVE alpha*xl, and a PE indicator-matmul
scatters [a | msg] into per-block PSUM accumulators.  All tiles of block b
use PE row band 32*(b%4), so xr is read straight out of the resident
projection tile (no replication DMAs).

bf16 everywhere on the edge path; accumulations (PSUM, h residual) fp32.
Self-contained: takes full (unsharded) inputs, returns the full output.
"""

import os
import numpy as np
from contextlib import ExitStack

import ml_dtypes
import concourse.bass as bass
import concourse.tile as tile
from concourse import bacc, mybir
from concourse.bass_utils import run_bass_kernel_spmd

# Problem constants (from the nn_GATv2Model spec)
N = 10000
E = 320000
F_IN = 7
D = 256
H = 8
C = 32
L = 2
G = 16
SLOPE = 0.2

NCORES = 8
NPC_REAL = 1250          # real nodes per core
NPC = 1280               # padded nodes per core (multiple of 256)
NP = NCORES * NPC        # padded total nodes
BLK = 32                 # dst nodes per block (indicator-matmul segment group)
NBLK = NPC // BLK        # blocks per core
PT = 128                 # edges per tile (partition dim)
CH = 5                   # AllGather chunks (NPC/256)
CHN = 256                # nodes per chunk
QW = 4                   # tiles per PSUM quad-group
DA = 8 + D               # scatter payload cols: 8 exp(e) + 256 msg
DG = D + 8               # gathered row: 256 xl + 8 folded source-linear
DPAD = 384               # xl_d row pitch in bf16 elems (768B, 256B multiple)

F32 = mybir.dt.float32
BF16 = mybir.dt.bfloat16
I16 = mybir.dt.int16
NBF = ml_dtypes.bfloat16


def _pad_map():
    n = np.arange(N)
    return (n // NPC_REAL) * NPC + (n % NPC_REAL)


def _prep_edges(edge_index: np.ndarray):
    """LPT-pack dst nodes into blocks, sort/shard/pad edges.

    Returns (t_blk, perms, cores) where perms[c] is old_of_new (new local
    id -> old padded-local id) and cores[c] has srcw/ind_pm/indt_pm."""
    pm = _pad_map()
    src_p = pm[edge_index[0]]
    dst_p = pm[edge_index[1]]
    owner = dst_p // NPC

    # ---- per-core LPT bin-packing of dst nodes into NBLK bins of 32 ----
    perms_new = []      # new_of_old per core
    perms_old = []      # old_of_new per core
    core_sel = []
    t_blk = 1
    for c in range(NCORES):
        sel = owner == c
        core_sel.append(sel)
        dl = dst_p[sel] - c * NPC
        deg = np.bincount(dl, minlength=NPC)
        order = np.argsort(-deg, kind="stable")
        binload = np.zeros(NBLK, np.int64)
        bincnt = np.zeros(NBLK, np.int64)
        assign = np.zeros(NPC, np.int64)
        for node in order:
            open_b = np.nonzero(bincnt < BLK)[0]
            b = open_b[np.argmin(binload[open_b])]
            assign[node] = b
            binload[b] += deg[node]
            bincnt[b] += 1
        new_of_old = np.zeros(NPC, np.int64)
        pos = np.zeros(NBLK, np.int64)
        for old in range(NPC):
            b = assign[old]
            new_of_old[old] = b * BLK + pos[b]
            pos[b] += 1
        perms_new.append(new_of_old)
        perms_old.append(np.argsort(new_of_old))
        t_blk = max(t_blk, int(np.ceil(binload.max() / PT)))
    t_blk = ((t_blk + QW - 1) // QW) * QW     # full q-groups

    # global chunk-major row map (256-node AllGather chunks): padded id ->
    # xl_d row
    row_of = np.zeros(NP, np.int64)
    for c in range(NCORES):
        r = perms_new[c]                      # old local -> new local
        j = r // CHN
        row_of[c * NPC:(c + 1) * NPC] = (j * NCORES + c) * CHN + (r % CHN)

    t8 = t_blk * 8
    ne_slots = NBLK * t_blk * PT
    cores = []
    for c in range(NCORES):
        sel = core_sel[c]
        es = row_of[src_p[sel]]                       # global xl_d rows
        ed = perms_new[c][dst_p[sel] - c * NPC]       # new local dst
        order = np.argsort(ed, kind="stable")
        es, ed = es[order], ed[order]
        blk = ed // BLK
        cnt = np.bincount(blk, minlength=NBLK)
        starts = np.zeros(NBLK, dtype=np.int64)
        starts[1:] = np.cumsum(cnt)[:-1]
        within = np.arange(len(es)) - starts[blk]
        # quad-interleaved slot order: slot of (block b = 4*qd+k4, within w)
        # lives at ((qd*t_blk + w//128)*4 + k4)*128 + w%128
        qdv, k4v = blk // 4, blk % 4
        slot = ((qdv * t_blk + within // PT) * 4 + k4v) * PT + within % PT

        src_slots = np.zeros(ne_slots, dtype=np.int64)
        dl32 = np.full(ne_slots, -1, dtype=np.int64)
        src_slots[slot] = es
        dl32[slot] = ed - blk * BLK

        # wrapped gather indices [128, ne_slots//16]: one 512-idx gather per
        # (qd, tt); slots are globally slot-major so the wrap is one reshape
        w16 = src_slots.reshape(-1, 16).T.astype(np.int16)   # [16, ne/16]
        srcw = np.tile(np.ascontiguousarray(w16), (8, 1))

        NQ = NBLK // 4
        ind = np.zeros((ne_slots, BLK), dtype=np.float32)
        valid = dl32 >= 0
        ind[np.nonzero(valid)[0], dl32[valid]] = 1.0
        ind = ind.reshape(NQ, t_blk, 4, PT, BLK)
        # scatter indicator per (qd, tt, k): ind_pm[128, qd, tt, k, 32]
        ind_pm = np.ascontiguousarray(
            ind.transpose(3, 0, 1, 2, 4)).astype(NBF)
        # transposed indicator: tile (qd, tt, k) at row band 32k:
        # indt_pm[32k:32k+32, qd, tt, :] = ind[qd, tt, k].T
        indt_pm = np.zeros((128, NQ, t_blk, PT), dtype=np.float32)
        for q in range(NQ):
            for tt in range(t_blk):
                for k4 in range(4):
                    indt_pm[32 * k4:32 * (k4 + 1), q, tt, :] = \
                        ind[q, tt, k4].T
        cores.append({"srcw": srcw, "ind_pm": ind_pm,
                      "indt_pm": indt_pm.astype(NBF)})
    return t_blk, perms_old, perms_new, row_of, cores


def _build(t_blk: int, debug: bool = False):
    """Build + compile the SPMD Bass program (identical on all cores)."""
    STAGE = int(os.environ.get("GAT_STAGE", "99"))
    GBUFS = int(os.environ.get("GAT_GBUFS", "3"))
    EESPLIT = os.environ.get("GAT_EESPLIT", "0") == "1"
    nt = NBLK * t_blk
    t8 = t_blk * 8

    nc = bacc.Bacc("TRN2", target_bir_lowering=False, debug=False,
                   num_devices=NCORES, num_swdge_queues=4)

    # ---- external inputs ----
    xaug = nc.dram_tensor("xaug", [F_IN + 1, NP], BF16, kind="ExternalInput").ap()
    xtown = nc.dram_tensor("xtown", [F_IN + 1, NPC], F32, kind="ExternalInput").ap()
    xtownb = nc.dram_tensor("xtownb", [F_IN + 1, NPC], BF16, kind="ExternalInput").ap()
    wr0f = nc.dram_tensor("wr0f", [F_IN + 1, D], BF16, kind="ExternalInput").ap()
    wp = nc.dram_tensor("wp", [F_IN + 1, D], F32, kind="ExternalInput").ap()
    wf0 = nc.dram_tensor("wf0", [F_IN + 1, DG], BF16, kind="ExternalInput").ap()
    wl1 = nc.dram_tensor("wl1", [128, 2, DG], BF16, kind="ExternalInput").ap()
    wr = nc.dram_tensor("wr", [128, 2, D], BF16, kind="ExternalInput").ap()
    attrep = nc.dram_tensor("attrep", [128, L, QW, D], BF16, kind="ExternalInput").ap()
    bcols = nc.dram_tensor("bcols", [128, 8], F32, kind="ExternalInput").ap()
    i128 = nc.dram_tensor("i128", [128, 128], F32, kind="ExternalInput").ap()
    i128b = nc.dram_tensor("i128b", [128, 128], BF16, kind="ExternalInput").ap()
    epsc = nc.dram_tensor("epsc", [128, 1], F32, kind="ExternalInput").ap()
    NQ = NBLK // 4
    srcw_d = nc.dram_tensor("srcw", [128, NBLK * t8], I16, kind="ExternalInput").ap()
    ind_d = nc.dram_tensor("ind", [128, NQ, t_blk, 4, BLK], BF16,
                           kind="ExternalInput").ap()
    indt_d = nc.dram_tensor("indt", [128, NQ, t_blk, PT], BF16,
                            kind="ExternalInput").ap()

    # ---- internal DRAM ----
    warm_d = nc.dram_tensor("warm_d", [2, DPAD], BF16).ap()
    xl_d0 = nc.dram_tensor("xl_d0", [NP, DPAD], BF16).ap()
    stage = nc.dram_tensor("stage", [NPC, DG], BF16).ap()
    xl_d1p = nc.dram_tensor("xl_d1p", [NP, DG], BF16, addr_space="Shared").ap()
    xl_d1 = nc.dram_tensor("xl_d1", [NP, DPAD], BF16).ap()

    # ---- outputs ----
    y_out = nc.dram_tensor("y_out", [1, NPC], F32, kind="ExternalOutput").ap()
    if debug:
        hdump = nc.dram_tensor("hdump", [L, 2, 128, NPC], F32, kind="ExternalOutput").ap()

    with tile.TileContext(nc) as tc, ExitStack() as ctx:
        cpool = ctx.enter_context(tc.tile_pool(name="consts", bufs=1))
        drain = ctx.enter_context(tc.tile_pool(name="drain", bufs=4))
        gath = ctx.enter_context(tc.tile_pool(name="gath", bufs=GBUFS))
        indp = ctx.enter_context(tc.tile_pool(name="indp", bufs=4))
        indtp = ctx.enter_context(tc.tile_pool(name="indtp", bufs=4))
        upool = ctx.enter_context(tc.tile_pool(name="upool", bufs=4))
        appool = ctx.enter_context(tc.tile_pool(name="appool", bufs=4))
        epool = ctx.enter_context(tc.tile_pool(name="epool", bufs=4))
        astp = ctx.enter_context(tc.tile_pool(name="astp", bufs=4))
        smallp = ctx.enter_context(tc.tile_pool(name="smallp", bufs=3))
        outbp = ctx.enter_context(tc.tile_pool(name="outbp", bufs=3))

        # PSUM (8 banks): pfeat 4 (one [128,4,512] tile), pbig 3 (projection
        # psum + the 4-block col-tiled scatter accumulators), ptr 1
        pfeat = ctx.enter_context(tc.tile_pool(name="pfeat", bufs=1, space="PSUM"))
        pbig = ctx.enter_context(tc.tile_pool(name="pbig", bufs=3, space="PSUM"))
        ptrp = ctx.enter_context(tc.tile_pool(name="ptrp", bufs=1, space="PSUM"))

        def pmm_tile():
            ps = pbig.tile([128, 320], F32, tag="pb", name="pmm")
            return ps

        # ---- resident SBUF constants ----
        x_sb = cpool.tile([F_IN + 1, NP], BF16)
        nc.sync.dma_start(x_sb[:], xaug)
        xto_sb = cpool.tile([F_IN + 1, NPC], F32)
        nc.sync.dma_start(xto_sb[:], xtown)
        xtob_sb = cpool.tile([F_IN + 1, NPC], BF16)
        nc.sync.dma_start(xtob_sb[:], xtownb)
        wr0f_sb = cpool.tile([F_IN + 1, D], BF16)
        nc.sync.dma_start(wr0f_sb[:], wr0f)
        wp_sb = cpool.tile([F_IN + 1, D], F32)
        nc.sync.dma_start(wp_sb[:], wp)
        wf0_sb = cpool.tile([F_IN + 1, DG], BF16)
        nc.sync.dma_start(wf0_sb[:], wf0)
        wl1_sb = cpool.tile([128, 2, DG], BF16)
        nc.sync.dma_start(wl1_sb[:], wl1)
        wr_sb = cpool.tile([128, 2, D], BF16)
        nc.sync.dma_start(wr_sb[:], wr)
        att_sb = cpool.tile([128, L, QW, D], BF16)
        nc.sync.dma_start(att_sb[:], attrep)
        bc_sb = cpool.tile([128, 8], F32)
        nc.sync.dma_start(bc_sb[:], bcols)
        i128_sb = cpool.tile([128, 128], F32)
        nc.sync.dma_start(i128_sb[:], i128)
        i128b_sb = cpool.tile([128, 128], BF16)
        nc.sync.dma_start(i128b_sb[:], i128b)
        eps_sb = cpool.tile([128, 1], F32)
        nc.sync.dma_start(eps_sb[:], epsc)
        srcw_sb = cpool.tile([128, NBLK * t8], I16)
        nc.sync.dma_start(srcw_sb[:], srcw_d)

        h_own = cpool.tile([128, 2, NPC], F32)     # h own slice, resident
        # xr projections, both layers, resident (band b%4 reads rows directly)
        xr_all = cpool.tile([128, L, NBLK // 4, D], BF16)

        nidx_reg = nc.gpsimd.to_reg(4 * PT)        # hoisted gather count reg

        # warmup: a throwaway gather so the SWDGE ucode activation cost
        # (~4.5us) is paid during the prologue, not the first sweep quad
        wix = cpool.tile([128, 8], I16)
        nc.vector.memset(wix[:], 0)
        warm = cpool.tile([128, 1, DPAD], BF16)
        nc.gpsimd.dma_gather(
            out_ap=warm[:], in_ap=warm_d,
            idxs_ap=wix[:], num_idxs=128, num_idxs_reg=128,
            elem_size=DPAD, queue_num=0)

        di = [0]

        def drain_ps(dst_ap, ps_ap):
            """PSUM -> SBUF copy alternating ACT/DVE to balance load."""
            if di[0] % 2 == 0:
                nc.scalar.activation(dst_ap, ps_ap,
                                     mybir.ActivationFunctionType.Copy)
            else:
                nc.vector.tensor_copy(dst_ap, ps_ap)
            di[0] += 1

        # ---- xl0 for ALL nodes straight from x (contraction dim 8),
        #      4 tiles per DRAM write ----
        for t4 in range(NP // 512 if STAGE >= 1 else 0):
            xw = drain.tile([128, 4, DG], BF16, tag="xw")
            for k in range(4):
                ps = pmm_tile()
                t = 4 * t4 + k
                nc.tensor.matmul(ps[:, 0:DG], x_sb[:, 128 * t:128 * (t + 1)],
                                 wf0_sb[:], start=True, stop=True)
                drain_ps(xw[:, k, :], ps[:, 0:DG])
            nc.sync.dma_start(
                xl_d0[512 * t4:512 * (t4 + 1), 0:DG]
                .rearrange("(f p) d -> p f d", p=128), xw[:])

        # ---- P0b: h0_own = [x|1] @ [Wp;bp] (feature-major, fp32) ----
        for k in range(4):
            for ch in range(2):
                ps = pmm_tile()
                nc.tensor.matmul(ps[:, 0:320], wp_sb[:, 128 * ch:128 * (ch + 1)],
                                 xto_sb[:, 320 * k:320 * (k + 1)],
                                 start=True, stop=True)
                drain_ps(h_own[:, ch, 320 * k:320 * (k + 1)], ps[:, 0:320])

        # ---- layers ----
        for l in range(L if STAGE >= 2 else 0):
            if l == 0:
                # xr0 = [x|1] @ (Wpb@Wr0): rank-8 bf16 matmul straight from
                # the core's own x -- no dependence on h0
                for jo in range(10 if STAGE >= 3 else 0):
                    ps = pmm_tile()
                    nc.tensor.matmul(ps[:, 0:D],
                                     xtob_sb[:, 128 * jo:128 * (jo + 1)],
                                     wr0f_sb[:], start=True, stop=True)
                    drain_ps(xr_all[:, l, jo, :], ps[:, 0:D])

            # edge sweep: quads of 4 blocks in lockstep, tile by tile; the
            # 4 subtiles of a tile-quad use PE row bands 32k (expand) and
            # col bands 32k (scatter), so LDWEIGHTS overlaps MATMUL
            nq_run = NQ if STAGE >= 4 else 0
            for qd in range(nq_run):
                # one buffer tile per quad: all 8 gathers' WAR deps clear at
                # once when the previous quad retires, so they arrive at the
                # GpSimd queue together and one ucode activation batches them
                xq = gath.tile([128, t_blk, 4, DPAD], BF16, tag="g")
                for tt in range(t_blk):
                    gi = qd * t_blk + tt
                    nc.gpsimd.dma_gather(
                        out_ap=xq[:, tt, :, :],
                        in_ap=xl_d0 if l == 0 else xl_d1,
                        idxs_ap=srcw_sb[:, gi * 32:(gi + 1) * 32],
                        num_idxs=4 * PT,
                        num_idxs_reg=nidx_reg,
                        elem_size=DPAD,
                        queue_num=gi % 4,
                    )
                xs = [xq[:, tt, :, :] for tt in range(t_blk)]
                ind_sb = indp.tile([128, t_blk, 4, BLK], BF16)
                nc.sync.dma_start(ind_sb[:], ind_d[:, qd, :, :, :])
                indt_sb = indtp.tile([128, t_blk, PT], BF16)
                nc.sync.dma_start(indt_sb[:], indt_d[:, qd, :, :])

                pb4 = pbig.tile([128, 320], F32, tag="pb")

                for tt in range(t_blk):
                    xsrc = xs[tt]
                    pf = pfeat.tile([128, QW, 512], F32, tag="pf")
                    # preload xl into PSUM: identity matmuls on the PE (it
                    # has slack; the band matmuls below run concurrently)
                    for k in range(4):
                        nc.tensor.matmul(pf[:, k, 0:D], i128b_sb[:],
                                         xsrc[:, k, 0:D],
                                         start=True, stop=False)
                        nc.tensor.matmul(pf[:, k, 0:D],
                                         indt_sb[32 * k:32 * (k + 1), tt, :],
                                         xr_all[32 * k:32 * (k + 1), l, qd, :],
                                         start=False, stop=True,
                                         skip_group_check=True,
                                         tile_position=(32 * k, 0))
                    u = upool.tile([128, QW, D], BF16)
                    nc.scalar.activation(u[:], pf[:, 0:4, 0:D],
                                         mybir.ActivationFunctionType.Abs)
                    ap_t = appool.tile([128, QW, D], BF16)
                    nc.vector.tensor_mul(ap_t[:], u[:], att_sb[:, l, :, :])
                    # per-head reduce as a 2-level 2x-mode tree + short reduce
                    apv = ap_t[:].rearrange("p a (h c) -> p a h c", h=H)
                    t1 = epool.tile([128, QW, H, 16], BF16, tag="t1")
                    nc.vector.tensor_add(t1[:], apv[:, :, :, 0:16],
                                         apv[:, :, :, 16:32])
                    e_sb = epool.tile([128, QW, 8], F32, tag="e")
                    with nc.allow_low_precision("bf16 partial sums"):
                        nc.vector.tensor_reduce(
                            e_sb[:], t1[:],
                            axis=mybir.AxisListType.X, op=mybir.AluOpType.add)
                    nc.vector.tensor_add(e_sb[:], e_sb[:], xsrc[:, :, D:DG])
                    ast = astp.tile([128, QW, DA], BF16)
                    nc.scalar.activation(ast[:, :, 0:8], e_sb[:],
                                         mybir.ActivationFunctionType.Exp)
                    if EESPLIT:
                        # expand exp per-head on ACT (which has slack with
                        # the preload on PE) so the DVE mul runs in 2x mode
                        eexp = epool.tile([128, QW, D], BF16, tag="ee")
                        nc.scalar.activation(
                            eexp[:].rearrange("p a (h c) -> p a h c", h=H),
                            e_sb[:].unsqueeze(-1).broadcast_to([128, QW, H, C]),
                            mybir.ActivationFunctionType.Exp)
                        nc.vector.tensor_mul(ast[:, :, 8:], xsrc[:, :, 0:D],
                                             eexp[:])
                    else:
                        nc.vector.tensor_mul(
                            ast[:, :, 8:].rearrange("p a (h c) -> p a h c", h=H),
                            xsrc[:, :, 0:D].rearrange("p a (h c) -> p a h c", h=H),
                            ast[:, :, 0:8].unsqueeze(-1).broadcast_to(
                                [128, 4, H, C]))
                    for k in range(4):
                        nc.tensor.matmul(pb4[32 * k:32 * (k + 1), 0:DA],
                                         ind_sb[:, tt, k, :], ast[:, k, :],
                                         start=(tt == 0), stop=(tt == t_blk - 1),
                                         skip_group_check=True,
                                         tile_position=(0, 32 * k))

                # 4 block epilogues into one node-major quad tile, then a
                # single pair of [128,128] transposes + h update
                hq = outbp.tile([128, D], F32)
                for k in range(4):
                    cj = 32 * k
                    pb = pb4[cj:cj + 32, 0:DA]
                    dsb = smallp.tile([128, 8], F32, tag="dsb")
                    nc.scalar.activation(dsb[cj:cj + 32, :], pb[:, 0:8],
                                         mybir.ActivationFunctionType.Identity,
                                         bias=eps_sb[cj:cj + 32, :])
                    dinv = smallp.tile([128, 8], F32, tag="dinv")
                    nc.vector.reciprocal(dinv[cj:cj + 32, :], dsb[cj:cj + 32, :])
                    nc.vector.tensor_mul(
                        hq[cj:cj + 32, :].rearrange("p (h c) -> p h c", h=H),
                        pb[:, 8:].rearrange("p (h c) -> p h c", h=H),
                        dinv[cj:cj + 32, :].unsqueeze(-1).broadcast_to([32, H, C]))
                pt = ptrp.tile([128, 2, 128], F32)
                for ch in range(2):
                    nc.tensor.transpose(pt[:, ch, :],
                                        hq[:, 128 * ch:128 * (ch + 1)],
                                        i128_sb[:])
                for ch in range(2):
                    nc.vector.tensor_add(
                        h_own[:, ch, 128 * qd:128 * (qd + 1)],
                        h_own[:, ch, 128 * qd:128 * (qd + 1)],
                        pt[:, ch, :])

                # after every quad in layer 0: bconv the finished 128 own
                # nodes, project through Wl1 (stage + AllGather chunk) and
                # Wr[1] (xr for layer 1)
                n0 = 128 * qd
                if l == 0 and STAGE >= 5:
                    for ch in range(2):
                        nc.vector.tensor_scalar_add(
                            h_own[:, ch, n0:n0 + 128],
                            h_own[:, ch, n0:n0 + 128],
                            bc_sb[:, 2 * l + ch:2 * l + ch + 1])
                    hb = drain.tile([128, 2, 128], BF16, tag="hb")
                    drain_ps(hb[:], h_own[:, :, n0:n0 + 128])
                    ps = pmm_tile()
                    for ch in range(2):
                        nc.tensor.matmul(ps[:, 0:DG], hb[:, ch, :],
                                         wl1_sb[:, ch, :],
                                         start=(ch == 0), stop=(ch == 1))
                    stg = drain.tile([128, DG], BF16, tag="stg")
                    drain_ps(stg[:], ps[:, 0:DG])
                    nc.sync.dma_start(stage[n0:n0 + 128, :], stg[:])
                    ps2 = pmm_tile()
                    for ch in range(2):
                        nc.tensor.matmul(ps2[:, 0:D], hb[:, ch, :],
                                         wr_sb[:, ch, :],
                                         start=(ch == 0), stop=(ch == 1))
                    drain_ps(xr_all[:, 1, qd, :], ps2[:, 0:D])
                    if qd % 2 == 1:
                        # AllGather packed 528B rows (31% fewer bytes than
                        # the padded gather pitch), then re-pitch locally
                        j = qd // 2
                        g0 = NCORES * CHN * j
                        nc.gpsimd.collective_compute(
                            "AllGather", mybir.AluOpType.bypass,
                            replica_groups=[list(range(NCORES))],
                            ins=[stage[CHN * j:CHN * (j + 1), :]],
                            outs=[xl_d1p[g0:g0 + NCORES * CHN, :]])
                        nc.sync.dma_start(
                            xl_d1[g0:g0 + NCORES * CHN, 0:DG],
                            xl_d1p[g0:g0 + NCORES * CHN, :])
                if l == 1 and STAGE >= 5:
                    # incremental final: bconv + y for the finished chunk
                    for ch in range(2):
                        nc.vector.tensor_scalar_add(
                            h_own[:, ch, n0:n0 + 128],
                            h_own[:, ch, n0:n0 + 128],
                            bc_sb[:, 2 * l + ch:2 * l + ch + 1])
                    ps = pmm_tile()
                    for ch in range(2):
                        nc.tensor.matmul(ps[0:1, 0:128], bc_sb[:, 4 + ch:5 + ch],
                                         h_own[:, ch, n0:n0 + 128],
                                         start=(ch == 0), stop=(ch == 1))
                    ysb = drain.tile([1, 128], F32, tag="ysb")
                    nc.scalar.activation(ysb[0:1, :], ps[0:1, 0:128],
                                         mybir.ActivationFunctionType.Copy)
                    nc.sync.dma_start(y_out[0:1, n0:n0 + 128], ysb[0:1, :])

            if STAGE < 5:
                # bconv for the whole slice at once
                for ch in range(2):
                    nc.vector.tensor_scalar_add(
                        h_own[:, ch, :], h_own[:, ch, :],
                        bc_sb[:, 2 * l + ch:2 * l + ch + 1])

            if debug:
                nc.sync.dma_start(hdump[l].rearrange("c p n -> p c n"), h_own[:])

        if STAGE < 5:
            # ---- final: y = h_own @ Wpred (non-incremental fallback) ----
            for k in range(4):
                w = 320
                ps = pmm_tile()
                for ch in range(2):
                    nc.tensor.matmul(ps[0:1, 0:w], bc_sb[:, 4 + ch:5 + ch],
                                     h_own[:, ch, 320 * k:320 * k + w],
                                     start=(ch == 0), stop=(ch == 1))
                ysb = drain.tile([1, 320], F32, tag="ysb")
                nc.scalar.activation(ysb[0:1, 0:w], ps[0:1, 0:w],
                                     mybir.ActivationFunctionType.Copy)
                nc.sync.dma_start(y_out[0:1, 320 * k:320 * k + w], ysb[0:1, 0:w])

    nc.compile()
    return nc


def _host_inputs(x, Wp, bp, Wl, Wr, att, bconv, Wpred, row_of, perms_old):
    pm = _pad_map()
    xp = np.zeros((NP, F_IN), dtype=np.float32)
    xp[pm] = np.asarray(x, dtype=np.float32)

    # augmented per-layer projections [Wl | Wl@A], A = 0.6*att blockdiag
    wla = []
    for l in range(L):
        A = np.zeros((D, H), dtype=np.float64)
        for hh in range(H):
            A[hh * C:(hh + 1) * C, hh] = 0.6 * att[l][hh]
        wla.append(np.concatenate([Wl[l], Wl[l] @ A], axis=1))  # [256, 264]

    # layer-0: fold through Wp (x is only 7-dim + ones column for biases)
    wpb = np.concatenate([Wp, bp[None, :]], axis=0)             # [8, 256]
    wf0 = wpb @ wla[0]                                          # [8, 264]
    wr0f = wpb @ Wr[0]                                          # [8, 256]

    xaug = np.ones((F_IN + 1, NP), dtype=np.float32)
    xaug[:F_IN, :] = 0.0
    xaug[:F_IN, row_of] = xp.T                                  # chunk-major cols

    wl1_p = np.zeros((128, 2, DG), dtype=np.float32)
    wr_p = np.zeros((128, 2, D), dtype=np.float32)
    for ch in range(2):
        wl1_p[:, ch, :] = wla[1][128 * ch:128 * (ch + 1), :]
        wr_p[:, ch, :] = Wr[1][128 * ch:128 * (ch + 1), :]
    att_p = np.zeros((128, L, QW, D), dtype=np.float32)
    for l in range(L):
        al = (0.4 * np.asarray(att[l], np.float64)).reshape(H * C).astype(np.float32)
        for q in range(QW):
            att_p[:, l, q, :] = al[None, :]

    bcols = np.zeros((128, 8), dtype=np.float32)
    for ch in range(2):
        for l in range(L):
            bcols[:, 2 * l + ch] = bconv[l][128 * ch:128 * (ch + 1)]
        bcols[:, 4 + ch] = Wpred[128 * ch:128 * (ch + 1), 0]

    shared = {
        "xaug": xaug.astype(NBF),
        "wp": wpb.astype(np.float32),
        "wf0": wf0.astype(NBF),
        "wr0f": wr0f.astype(NBF),
        "wl1": wl1_p.astype(NBF), "wr": wr_p.astype(NBF),
        "attrep": att_p.astype(NBF),
        "bcols": bcols,
        "i128": np.eye(128, dtype=np.float32),
        "i128b": np.eye(128, dtype=np.float32).astype(NBF),
        "epsc": np.full((128, 1), 1e-16, dtype=np.float32),
    }
    xtowns = []
    for c in range(NCORES):
        xt = np.ones((F_IN + 1, NPC), dtype=np.float32)
        xt[:F_IN, :] = xp[c * NPC + perms_old[c]].T
        xtowns.append(xt)
    return shared, xtowns


_CACHE = {}


def kernel(x, edge_index, batch, Wp, bp, Wl, Wr, att, bconv, Wpred, bpred,
           debug=False, _timing=None):
    x = np.asarray(x)
    edge_index = np.asarray(edge_index).astype(np.int64)
    batch = np.asarray(batch).astype(np.int64)

    t_blk, perms_old, perms_new, row_of, cores = _prep_edges(edge_index)
    shared, xtowns = _host_inputs(np.asarray(x), np.asarray(Wp), np.asarray(bp),
                                  np.asarray(Wl), np.asarray(Wr), np.asarray(att),
                                  np.asarray(bconv), np.asarray(Wpred),
                                  row_of, perms_old)

    key = (t_blk, bool(debug), os.environ.get("GAT_STAGE", "99"),
           os.environ.get("GAT_GBUFS", "3"), os.environ.get("GAT_EESPLIT", "0"))
    if key not in _CACHE:
        _CACHE[key] = _build(t_blk, debug=debug)
    nc = _CACHE[key]

    in_maps = []
    for c in range(NCORES):
        m = dict(shared)
        m["xtown"] = xtowns[c]
        m["xtownb"] = xtowns[c].astype(NBF)
        m["srcw"] = cores[c]["srcw"]
        m["ind"] = cores[c]["ind_pm"]
        m["indt"] = cores[c]["indt_pm"]
        in_maps.append(m)

    kw = {k: v for k, v in (_timing or {}).items() if k != "result"}
    res = run_bass_kernel_spmd(nc, in_maps, list(range(NCORES)), **kw)
    if _timing is not None:
        _timing["result"] = res

    pm = _pad_map()
    ycat = np.stack([res.results[c]["y_out"][0] for c in range(NCORES)])
    p = pm
    c_of = p // NPC
    r_new = np.concatenate([perms_new[c][None] for c in range(NCORES)])  # [8, NPC]
    y_real = ycat[c_of, r_new[c_of, p % NPC]]
    sums = np.bincount(batch, weights=y_real.astype(np.float64), minlength=G)
    cnt = np.bincount(batch, minlength=G).astype(np.float64)
    out = sums / np.maximum(cnt, 1.0) + float(np.asarray(bpred).reshape(-1)[0])
    if debug:
        return out.astype(np.float32)[:, None], res
    return out.astype(np.float32)[:, None]


# revision 67
# speedup vs baseline: 1.0169x; 1.0169x over previous
"""GATv2 (2-layer, 8-head) message-passing kernel for 8 Trainium2 NeuronCores.

Sharding: nodes (and incoming edges) are partitioned across 8 cores by
destination; weights replicated.  Per core, dst nodes are LPT-bin-packed into
40 blocks of 32 nodes (balancing edge counts so every block needs the same
t_blk=8 tiles of 128 edges).  Layer-0 source features xl0 are computed
replicated directly from x via the host-folded weight [Wp;bp]@Wl0
(contraction dim 8), written to DRAM (512B rows), and gathered per edge with
SWDGE.  Layer-1 xl is projected per chunk of 256 own nodes as layer-0 blocks
retire and a chunked AllGather distributes it, overlapping the collective
with compute.

SWDGE descriptor generation has a ~4.4us per-ucode-activation overhead but
batches all pending gather instructions into one activation, so the gather
pool is kept deep (6 bufs x 2 halves) to keep many gathers pending; that
drops the effective cost from ~9ns/edge to ~3ns/edge.

Edge math per 128-edge tile: one batched ACT copy preloads the gathered xl
of a whole 4-tile q-group into PSUM, one PE matmul per tile accumulates the
indicator-expanded xr on top (feat = xl[src]+xr[dst]); ACT Lrelu, DVE mul by
att + per-head reduce, ACT exp, DVE alpha*xl, and a PE indicator-matmul
scatters [a | msg] into per-block PSUM accumulators.  All tiles of block b
use PE row band 32*(b%4), so xr is read straight out of the resident
projection tile (no replication DMAs).

bf16 everywhere on the edge path; accumulations (PSUM, h residual) fp32.
Self-contained: takes full (unsharded) inputs, returns the full output.
"""

import os
import numpy as np
from contextlib import ExitStack

import ml_dtypes
import concourse.bass as bass
import concourse.tile as tile
from concourse import bacc, mybir
from concourse.bass_utils import run_bass_kernel_spmd

# Problem constants (from the nn_GATv2Model spec)
N = 10000
E = 320000
F_IN = 7
D = 256
H = 8
C = 32
L = 2
G = 16
SLOPE = 0.2

NCORES = 8
NPC_REAL = 1250          # real nodes per core
NPC = 1280               # padded nodes per core (multiple of 256)
NP = NCORES * NPC        # padded total nodes
BLK = 32                 # dst nodes per block (indicator-matmul segment group)
NBLK = NPC // BLK        # blocks per core
PT = 128                 # edges per tile (partition dim)
CH = 5                   # AllGather chunks (NPC/256)
CHN = 256                # nodes per chunk
QW = 4                   # tiles per PSUM quad-group
DA = 8 + D               # scatter payload cols: 8 exp(e) + 256 msg
DG = D + 8               # gathered row: 256 xl + 8 folded source-linear
DPAD = 384               # xl_d row pitch in bf16 elems (768B, 256B multiple)

F32 = mybir.dt.float32
BF16 = mybir.dt.bfloat16
I16 = mybir.dt.int16
NBF = ml_dtypes.bfloat16


def _pad_map():
    n = np.arange(N)
    return (n // NPC_REAL) * NPC + (n % NPC_REAL)


def _prep_edges(edge_index: np.ndarray):
    """LPT-pack dst nodes into blocks, sort/shard/pad edges.

    Returns (t_blk, perms, cores) where perms[c] is old_of_new (new local
    id -> old padded-local id) and cores[c] has srcw/ind_pm/indt_pm."""
    pm = _pad_map()
    src_p = pm[edge_index[0]]
    dst_p = pm[edge_index[1]]
    owner = dst_p // NPC

    # ---- per-core LPT bin-packing of dst nodes into NBLK bins of 32 ----
    perms_new = []      # new_of_old per core
    perms_old = []      # old_of_new per core
    core_sel = []
    t_blk = 1
    for c in range(NCORES):
        sel = owner == c
        core_sel.append(sel)
        dl = dst_p[sel] - c * NPC
        deg = np.bincount(dl, minlength=NPC)
        order = np.argsort(-deg, kind="stable")
        binload = np.zeros(NBLK, np.int64)
        bincnt = np.zeros(NBLK, np.int64)
        assign = np.zeros(NPC, np.int64)
        for node in order:
            open_b = np.nonzero(bincnt < BLK)[0]
            b = open_b[np.argmin(binload[open_b])]
            assign[node] = b
            binload[b] += deg[node]
            bincnt[b] += 1
        new_of_old = np.zeros(NPC, np.int64)
        pos = np.zeros(NBLK, np.int64)
        for old in range(NPC):
            b = assign[old]
            new_of_old[old] = b * BLK + pos[b]
            pos[b] += 1
        perms_new.append(new_of_old)
        perms_old.append(np.argsort(new_of_old))
        t_blk = max(t_blk, int(np.ceil(binload.max() / PT)))
    t_blk = ((t_blk + QW - 1) // QW) * QW     # full q-groups

    # global chunk-major row map (256-node AllGather chunks): padded id ->
    # xl_d row
    row_of = np.zeros(NP, np.int64)
    for c in range(NCORES):
        r = perms_new[c]                      # old local -> new local
        j = r // CHN
        row_of[c * NPC:(c + 1) * NPC] = (j * NCORES + c) * CHN + (r % CHN)

    t8 = t_blk * 8
    ne_slots = NBLK * t_blk * PT
    cores = []
    for c in range(NCORES):
        sel = core_sel[c]
        es = row_of[src_p[sel]]                       # global xl_d rows
        ed = perms_new[c][dst_p[sel] - c * NPC]       # new local dst
        order = np.argsort(ed, kind="stable")
        es, ed = es[order], ed[order]
        blk = ed // BLK
        cnt = np.bincount(blk, minlength=NBLK)
        starts = np.zeros(NBLK, dtype=np.int64)
        starts[1:] = np.cumsum(cnt)[:-1]
        within = np.arange(len(es)) - starts[blk]
        # quad-interleaved slot order: slot of (block b = 4*qd+k4, within w)
        # lives at ((qd*t_blk + w//128)*4 + k4)*128 + w%128
        qdv, k4v = blk // 4, blk % 4
        slot = ((qdv * t_blk + within // PT) * 4 + k4v) * PT + within % PT

        src_slots = np.zeros(ne_slots, dtype=np.int64)
        dl32 = np.full(ne_slots, -1, dtype=np.int64)
        src_slots[slot] = es
        dl32[slot] = ed - blk * BLK

        # wrapped gather indices [128, ne_slots//16]: one 512-idx gather per
        # (qd, tt); slots are globally slot-major so the wrap is one reshape
        w16 = src_slots.reshape(-1, 16).T.astype(np.int16)   # [16, ne/16]
        srcw = np.tile(np.ascontiguousarray(w16), (8, 1))

        NQ = NBLK // 4
        ind = np.zeros((ne_slots, BLK), dtype=np.float32)
        valid = dl32 >= 0
        ind[np.nonzero(valid)[0], dl32[valid]] = 1.0
        ind = ind.reshape(NQ, t_blk, 4, PT, BLK)
        # scatter indicator per (qd, tt, k): ind_pm[128, qd, tt, k, 32]
        ind_pm = np.ascontiguousarray(
            ind.transpose(3, 0, 1, 2, 4)).astype(NBF)
        # transposed indicator: tile (qd, tt, k) at row band 32k:
        # indt_pm[32k:32k+32, qd, tt, :] = ind[qd, tt, k].T
        indt_pm = np.zeros((128, NQ, t_blk, PT), dtype=np.float32)
        for q in range(NQ):
            for tt in range(t_blk):
                for k4 in range(4):
                    indt_pm[32 * k4:32 * (k4 + 1), q, tt, :] = \
                        ind[q, tt, k4].T
        cores.append({"srcw": srcw, "ind_pm": ind_pm,
                      "indt_pm": indt_pm.astype(NBF)})
    return t_blk, perms_old, perms_new, row_of, cores


def _build(t_blk: int, debug: bool = False):
    """Build + compile the SPMD Bass program (identical on all cores)."""
    STAGE = int(os.environ.get("GAT_STAGE", "99"))
    GBUFS = int(os.environ.get("GAT_GBUFS", "3"))
    EESPLIT = os.environ.get("GAT_EESPLIT", "0") == "1"
    nt = NBLK * t_blk
    t8 = t_blk * 8

    nc = bacc.Bacc("TRN2", target_bir_lowering=False, debug=False,
                   num_devices=NCORES, num_swdge_queues=4)

    # ---- external inputs ----
    xaug = nc.dram_tensor("xaug", [F_IN + 1, NP], BF16, kind="ExternalInput").ap()
    xtown = nc.dram_tensor("xtown", [F_IN + 1, NPC], F32, kind="ExternalInput").ap()
    xtownb = nc.dram_tensor("xtownb", [F_IN + 1, NPC], BF16, kind="ExternalInput").ap()
    wr0f = nc.dram_tensor("wr0f", [F_IN + 1, D], BF16, kind="ExternalInput").ap()
    wp = nc.dram_tensor("wp", [F_IN + 1, D], F32, kind="ExternalInput").ap()
    wf0 = nc.dram_tensor("wf0", [F_IN + 1, DG], BF16, kind="ExternalInput").ap()
    wl1 = nc.dram_tensor("wl1", [128, 2, DG], BF16, kind="ExternalInput").ap()
    wr = nc.dram_tensor("wr", [128, 2, D], BF16, kind="ExternalInput").ap()
    attrep = nc.dram_tensor("attrep", [128, L, QW, D], BF16, kind="ExternalInput").ap()
    bcols = nc.dram_tensor("bcols", [128, 8], F32, kind="ExternalInput").ap()
    i128 = nc.dram_tensor("i128", [128, 128], F32, kind="ExternalInput").ap()
    i128b = nc.dram_tensor("i128b", [128, 128], BF16, kind="ExternalInput").ap()
    epsc = nc.dram_tensor("epsc", [128, 1], F32, kind="ExternalInput").ap()
    NQ = NBLK // 4
    srcw_d = nc.dram_tensor("srcw", [128, NBLK * t8], I16, kind="ExternalInput").ap()
    ind_d = nc.dram_tensor("ind", [128, NQ, t_blk, 4, BLK], BF16,
                           kind="ExternalInput").ap()
    indt_d = nc.dram_tensor("indt", [128, NQ, t_blk, PT], BF16,
                            kind="ExternalInput").ap()

    # ---- internal DRAM ----
    warm_d = nc.dram_tensor("warm_d", [2, DPAD], BF16).ap()
    xl_d0 = nc.dram_tensor("xl_d0", [NP, DPAD], BF16).ap()
    stage = nc.dram_tensor("stage", [NPC, DG], BF16).ap()
    xl_d1p = nc.dram_tensor("xl_d1p", [NP, DG], BF16, addr_space="Shared").ap()
    xl_d1 = nc.dram_tensor("xl_d1", [NP, DPAD], BF16).ap()

    # ---- outputs ----
    y_out = nc.dram_tensor("y_out", [1, NPC], F32, kind="ExternalOutput").ap()
    if debug:
        hdump = nc.dram_tensor("hdump", [L, 2, 128, NPC], F32, kind="ExternalOutput").ap()

    with tile.TileContext(nc) as tc, ExitStack() as ctx:
        cpool = ctx.enter_context(tc.tile_pool(name="consts", bufs=1))
        drain = ctx.enter_context(tc.tile_pool(name="drain", bufs=4))
        gath = ctx.enter_context(tc.tile_pool(name="gath", bufs=GBUFS))
        indp = ctx.enter_context(tc.tile_pool(name="indp", bufs=4))
        indtp = ctx.enter_context(tc.tile_pool(name="indtp", bufs=4))
        upool = ctx.enter_context(tc.tile_pool(name="upool", bufs=4))
        appool = ctx.enter_context(tc.tile_pool(name="appool", bufs=4))
        epool = ctx.enter_context(tc.tile_pool(name="epool", bufs=4))
        astp = ctx.enter_context(tc.tile_pool(name="astp", bufs=4))
        smallp = ctx.enter_context(tc.tile_pool(name="smallp", bufs=3))
        outbp = ctx.enter_context(tc.tile_pool(name="outbp", bufs=3))

        # PSUM (8 banks): pfeat 4 (one [128,4,512] tile), pbig 3 (projection
        # psum + the 4-block col-tiled scatter accumulators), ptr 1
        pfeat = ctx.enter_context(tc.tile_pool(name="pfeat", bufs=1, space="PSUM"))
        pbig = ctx.enter_context(tc.tile_pool(name="pbig", bufs=3, space="PSUM"))
        ptrp = ctx.enter_context(tc.tile_pool(name="ptrp", bufs=1, space="PSUM"))

        def pmm_tile():
            ps = pbig.tile([128, 320], F32, tag="pb", name="pmm")
            return ps

        # ---- resident SBUF constants ----
        x_sb = cpool.tile([F_IN + 1, NP], BF16)
        nc.sync.dma_start(x_sb[:], xaug)
        xto_sb = cpool.tile([F_IN + 1, NPC], F32)
        nc.sync.dma_start(xto_sb[:], xtown)
        xtob_sb = cpool.tile([F_IN + 1, NPC], BF16)
        nc.sync.dma_start(xtob_sb[:], xtownb)
        wr0f_sb = cpool.tile([F_IN + 1, D], BF16)
        nc.sync.dma_start(wr0f_sb[:], wr0f)
        wp_sb = cpool.tile([F_IN + 1, D], F32)
        nc.sync.dma_start(wp_sb[:], wp)
        wf0_sb = cpool.tile([F_IN + 1, DG], BF16)
        nc.sync.dma_start(wf0_sb[:], wf0)
        wl1_sb = cpool.tile([128, 2, DG], BF16)
        nc.sync.dma_start(wl1_sb[:], wl1)
        wr_sb = cpool.tile([128, 2, D], BF16)
        nc.sync.dma_start(wr_sb[:], wr)
        att_sb = cpool.tile([128, L, QW, D], BF16)
        nc.sync.dma_start(att_sb[:], attrep)
        bc_sb = cpool.tile([128, 8], F32)
        nc.sync.dma_start(bc_sb[:], bcols)
        i128_sb = cpool.tile([128, 128], F32)
        nc.sync.dma_start(i128_sb[:], i128)
        i128b_sb = cpool.tile([128, 128], BF16)
        nc.sync.dma_start(i128b_sb[:], i128b)
        eps_sb = cpool.tile([128, 1], F32)
        nc.sync.dma_start(eps_sb[:], epsc)
        srcw_sb = cpool.tile([128, NBLK * t8], I16)
        nc.sync.dma_start(srcw_sb[:], srcw_d)

        h_own = cpool.tile([128, 2, NPC], F32)     # h own slice, resident
        # xr projections, both layers, resident (band b%4 reads rows directly)
        xr_all = cpool.tile([128, L, NBLK // 4, D], BF16)

        nidx_reg = nc.gpsimd.to_reg(4 * PT)        # hoisted gather count reg

        # warmup: a throwaway gather so the SWDGE ucode activation cost
        # (~4.5us) is paid during the prologue, not the first sweep quad
        wix = cpool.tile([128, 8], I16)
        nc.vector.memset(wix[:], 0)
        warm = cpool.tile([128, 1, DPAD], BF16)
        nc.gpsimd.dma_gather(
            out_ap=warm[:], in_ap=warm_d,
            idxs_ap=wix[:], num_idxs=128, num_idxs_reg=128,
            elem_size=DPAD, queue_num=0)

        di = [0]

        def drain_ps(dst_ap, ps_ap):
            """PSUM -> SBUF copy alternating ACT/DVE to balance load."""
            if di[0] % 2 == 0:
                nc.scalar.activation(dst_ap, ps_ap,
                                     mybir.ActivationFunctionType.Copy)
            else:
                nc.vector.tensor_copy(dst_ap, ps_ap)
            di[0] += 1

        # ---- xl0 for ALL nodes straight from x (contraction dim 8),
        #      4 tiles per DRAM write ----
        for t4 in range(NP // 512 if STAGE >= 1 else 0):
            xw = drain.tile([128, 4, DG], BF16, tag="xw")
            for k in range(4):
                ps = pmm_tile()
                t = 4 * t4 + k
                nc.tensor.matmul(ps[:, 0:DG], x_sb[:, 128 * t:128 * (t + 1)],
                                 wf0_sb[:], start=True, stop=True)
                drain_ps(xw[:, k, :], ps[:, 0:DG])
            nc.sync.dma_start(
                xl_d0[512 * t4:512 * (t4 + 1), 0:DG]
                .rearrange("(f p) d -> p f d", p=128), xw[:])

        # ---- P0b: h0_own = [x|1] @ [Wp;bp] (feature-major, fp32) ----
        for k in range(4):
            for ch in range(2):
                ps = pmm_tile()
                nc.tensor.matmul(ps[:, 0:320], wp_sb[:, 128 * ch:128 * (ch + 1)],
                                 xto_sb[:, 320 * k:320 * (k + 1)],
                                 start=True, stop=True)
                drain_ps(h_own[:, ch, 320 * k:320 * (k + 1)], ps[:, 0:320])

        # ---- layers ----
        for l in range(L if STAGE >= 2 else 0):
            if l == 0:
                # xr0 = [x|1] @ (Wpb@Wr0): rank-8 bf16 matmul straight from
                # the core's own x -- no dependence on h0
                for jo in range(10 if STAGE >= 3 else 0):
                    ps = pmm_tile()
                    nc.tensor.matmul(ps[:, 0:D],
                                     xtob_sb[:, 128 * jo:128 * (jo + 1)],
                                     wr0f_sb[:], start=True, stop=True)
                    drain_ps(xr_all[:, l, jo, :], ps[:, 0:D])

            # edge sweep: quads of 4 blocks in lockstep, tile by tile; the
            # 4 subtiles of a tile-quad use PE row bands 32k (expand) and
            # col bands 32k (scatter), so LDWEIGHTS overlaps MATMUL
            nq_run = NQ if STAGE >= 4 else 0
            for qd in range(nq_run):
                # one buffer tile per quad: all 8 gathers' WAR deps clear at
                # once when the previous quad retires, so they arrive at the
                # GpSimd queue together and one ucode activation batches them
                xq = gath.tile([128, t_blk, 4, DPAD], BF16, tag="g")
                for tt in range(t_blk):
                    gi = qd * t_blk + tt
                    nc.gpsimd.dma_gather(
                        out_ap=xq[:, tt, :, :],
                        in_ap=xl_d0 if l == 0 else xl_d1,
                        idxs_ap=srcw_sb[:, gi * 32:(gi + 1) * 32],
                        num_idxs=4 * PT,
                        num_idxs_reg=nidx_reg,
                        elem_size=DPAD,
                        queue_num=gi % 4,
                    )
                xs = [xq[:, tt, :, :] for tt in range(t_blk)]
                ind_sb = indp.tile([128, t_blk, 4, BLK], BF16)
                nc.sync.dma_start(ind_sb[:], ind_d[:, qd, :, :, :])
                indt_sb = indtp.tile([128, t_blk, PT], BF16)
                nc.sync.dma_start(indt_sb[:], indt_d[:, qd, :, :])

                pb4 = pbig.tile([128, 320], F32, tag="pb")

                for tt in range(t_blk):
                    xsrc = xs[tt]
                    pf = pfeat.tile([128, QW, 512], F32, tag="pf")
                    # batched preload of the whole tile-quad into PSUM
                    nc.scalar.activation(
                        pf[:, 0:4, 0:D], xsrc[:, :, 0:D],
                        mybir.ActivationFunctionType.Copy)
                    for k in range(4):
                        nc.tensor.matmul(pf[:, k, 0:D],
                                         indt_sb[32 * k:32 * (k + 1), tt, :],
                                         xr_all[32 * k:32 * (k + 1), l, qd, :],
                                         start=False, stop=True,
                                         skip_group_check=True,
                                         tile_position=(32 * k, 0))
                    u = upool.tile([128, QW, D], BF16)
                    nc.scalar.activation(u[:], pf[:, 0:4, 0:D],
                                         mybir.ActivationFunctionType.Abs)
                    ap_t = appool.tile([128, QW, D], BF16)
                    nc.vector.tensor_mul(ap_t[:], u[:], att_sb[:, l, :, :])
                    # per-head reduce as a 2-level 2x-mode tree + short reduce
                    apv = ap_t[:].rearrange("p a (h c) -> p a h c", h=H)
                    t1 = epool.tile([128, QW, H, 16], BF16, tag="t1")
                    nc.vector.tensor_add(t1[:], apv[:, :, :, 0:16],
                                         apv[:, :, :, 16:32])
                    e_sb = epool.tile([128, QW, 8], F32, tag="e")
                    with nc.allow_low_precision("bf16 partial sums"):
                        nc.vector.tensor_reduce(
                            e_sb[:], t1[:],
                            axis=mybir.AxisListType.X, op=mybir.AluOpType.add)
                    nc.vector.tensor_add(e_sb[:], e_sb[:], xsrc[:, :, D:DG])
                    ast = astp.tile([128, QW, DA], BF16)
                    nc.scalar.activation(ast[:, :, 0:8], e_sb[:],
                                         mybir.ActivationFunctionType.Exp)
                    if EESPLIT:
                        # expand exp per-head on ACT (which has slack with
                        # the preload on PE) so the DVE mul runs in 2x mode
                        eexp = epool.tile([128, QW, D], BF16, tag="ee")
                        nc.scalar.activation(
                            eexp[:].rearrange("p a (h c) -> p a h c", h=H),
                            e_sb[:].unsqueeze(-1).broadcast_to([128, QW, H, C]),
                            mybir.ActivationFunctionType.Exp)
                        nc.vector.tensor_mul(ast[:, :, 8:], xsrc[:, :, 0:D],
                                             eexp[:])
                    else:
                        nc.vector.tensor_mul(
                            ast[:, :, 8:].rearrange("p a (h c) -> p a h c", h=H),
                            xsrc[:, :, 0:D].rearrange("p a (h c) -> p a h c", h=H),
                            ast[:, :, 0:8].unsqueeze(-1).broadcast_to(
                                [128, 4, H, C]))
                    for k in range(4):
                        nc.tensor.matmul(pb4[32 * k:32 * (k + 1), 0:DA],
                                         ind_sb[:, tt, k, :], ast[:, k, :],
                                         start=(tt == 0), stop=(tt == t_blk - 1),
                                         skip_group_check=True,
                                         tile_position=(0, 32 * k))

                # 4 block epilogues into one node-major quad tile, then a
                # single pair of [128,128] transposes + h update
                hq = outbp.tile([128, D], F32)
                for k in range(4):
                    cj = 32 * k
                    pb = pb4[cj:cj + 32, 0:DA]
                    dsb = smallp.tile([128, 8], F32, tag="dsb")
                    nc.scalar.activation(dsb[cj:cj + 32, :], pb[:, 0:8],
                                         mybir.ActivationFunctionType.Identity,
                                         bias=eps_sb[cj:cj + 32, :])
                    dinv = smallp.tile([128, 8], F32, tag="dinv")
                    nc.vector.reciprocal(dinv[cj:cj + 32, :], dsb[cj:cj + 32, :])
                    nc.vector.tensor_mul(
                        hq[cj:cj + 32, :].rearrange("p (h c) -> p h c", h=H),
                        pb[:, 8:].rearrange("p (h c) -> p h c", h=H),
                        dinv[cj:cj + 32, :].unsqueeze(-1).broadcast_to([32, H, C]))
                pt = ptrp.tile([128, 2, 128], F32)
                for ch in range(2):
                    nc.tensor.transpose(pt[:, ch, :],
                                        hq[:, 128 * ch:128 * (ch + 1)],
                                        i128_sb[:])
                for ch in range(2):
                    nc.vector.tensor_add(
                        h_own[:, ch, 128 * qd:128 * (qd + 1)],
                        h_own[:, ch, 128 * qd:128 * (qd + 1)],
                        pt[:, ch, :])

                # after every quad in layer 0: bconv the finished 128 own
                # nodes, project through Wl1 (stage + AllGather chunk) and
                # Wr[1] (xr for layer 1)
                n0 = 128 * qd
                if l == 0 and STAGE >= 5:
                    for ch in range(2):
                        nc.vector.tensor_scalar_add(
                            h_own[:, ch, n0:n0 + 128],
                            h_own[:, ch, n0:n0 + 128],
                            bc_sb[:, 2 * l + ch:2 * l + ch + 1])
                    hb = drain.tile([128, 2, 128], BF16, tag="hb")
                    drain_ps(hb[:], h_own[:, :, n0:n0 + 128])
                    ps = pmm_tile()
                    for ch in range(2):
                        nc.tensor.matmul(ps[:, 0:DG], hb[:, ch, :],
                                         wl1_sb[:, ch, :],
                                         start=(ch == 0), stop=(ch == 1))
                    stg = drain.tile([128, DG], BF16, tag="stg")
                    drain_ps(stg[:], ps[:, 0:DG])
                    nc.sync.dma_start(stage[n0:n0 + 128, :], stg[:])
                    ps2 = pmm_tile()
                    for ch in range(2):
                        nc.tensor.matmul(ps2[:, 0:D], hb[:, ch, :],
                                         wr_sb[:, ch, :],
                                         start=(ch == 0), stop=(ch == 1))
                    drain_ps(xr_all[:, 1, qd, :], ps2[:, 0:D])
                    if qd % 2 == 1:
                        # AllGather packed 528B rows (31% fewer bytes than
                        # the padded gather pitch), then re-pitch locally
                        j = qd // 2
                        g0 = NCORES * CHN * j
                        nc.gpsimd.collective_compute(
                            "AllGather", mybir.AluOpType.bypass,
                            replica_groups=[list(range(NCORES))],
                            ins=[stage[CHN * j:CHN * (j + 1), :]],
                            outs=[xl_d1p[g0:g0 + NCORES * CHN, :]])
                        nc.sync.dma_start(
                            xl_d1[g0:g0 + NCORES * CHN, 0:DG],
                            xl_d1p[g0:g0 + NCORES * CHN, :])
                if l == 1 and STAGE >= 5:
                    # incremental final: bconv + y for the finished chunk
                    for ch in range(2):
                        nc.vector.tensor_scalar_add(
                            h_own[:, ch, n0:n0 + 128],
                            h_own[:, ch, n0:n0 + 128],
                            bc_sb[:, 2 * l + ch:2 * l + ch + 1])
                    ps = pmm_tile()
                    for ch in range(2):
                        nc.tensor.matmul(ps[0:1, 0:128], bc_sb[:, 4 + ch:5 + ch],
                                         h_own[:, ch, n0:n0 + 128],
                                         start=(ch == 0), stop=(ch == 1))
                    ysb = drain.tile([1, 128], F32, tag="ysb")
                    nc.scalar.activation(ysb[0:1, :], ps[0:1, 0:128],
                                         mybir.ActivationFunctionType.Copy)
                    nc.sync.dma_start(y_out[0:1, n0:n0 + 128], ysb[0:1, :])

            if STAGE < 5:
                # bconv for the whole slice at once
                for ch in range(2):
                    nc.vector.tensor_scalar_add(
                        h_own[:, ch, :], h_own[:, ch, :],
                        bc_sb[:, 2 * l + ch:2 * l + ch + 1])

            if debug:
                nc.sync.dma_start(hdump[l].rearrange("c p n -> p c n"), h_own[:])

        if STAGE < 5:
            # ---- final: y = h_own @ Wpred (non-incremental fallback) ----
            for k in range(4):
                w = 320
                ps = pmm_tile()
                for ch in range(2):
                    nc.tensor.matmul(ps[0:1, 0:w], bc_sb[:, 4 + ch:5 + ch],
                                     h_own[:, ch, 320 * k:320 * k + w],
                                     start=(ch == 0), stop=(ch == 1))
                ysb = drain.tile([1, 320], F32, tag="ysb")
                nc.scalar.activation(ysb[0:1, 0:w], ps[0:1, 0:w],
                                     mybir.ActivationFunctionType.Copy)
                nc.sync.dma_start(y_out[0:1, 320 * k:320 * k + w], ysb[0:1, 0:w])

    nc.compile()
    return nc


def _host_inputs(x, Wp, bp, Wl, Wr, att, bconv, Wpred, row_of, perms_old):
    pm = _pad_map()
    xp = np.zeros((NP, F_IN), dtype=np.float32)
    xp[pm] = np.asarray(x, dtype=np.float32)

    # augmented per-layer projections [Wl | Wl@A], A = 0.6*att blockdiag
    wla = []
    for l in range(L):
        A = np.zeros((D, H), dtype=np.float64)
        for hh in range(H):
            A[hh * C:(hh + 1) * C, hh] = 0.6 * att[l][hh]
        wla.append(np.concatenate([Wl[l], Wl[l] @ A], axis=1))  # [256, 264]

    # layer-0: fold through Wp (x is only 7-dim + ones column for biases)
    wpb = np.concatenate([Wp, bp[None, :]], axis=0)             # [8, 256]
    wf0 = wpb @ wla[0]                                          # [8, 264]
    wr0f = wpb @ Wr[0]                                          # [8, 256]

    xaug = np.ones((F_IN + 1, NP), dtype=np.float32)
    xaug[:F_IN, :] = 0.0
    xaug[:F_IN, row_of] = xp.T                                  # chunk-major cols

    wl1_p = np.zeros((128, 2, DG), dtype=np.float32)
    wr_p = np.zeros((128, 2, D), dtype=np.float32)
    for ch in range(2):
        wl1_p[:, ch, :] = wla[1][128 * ch:128 * (ch + 1), :]
        wr_p[:, ch, :] = Wr[1][128 * ch:128 * (ch + 1), :]
    att_p = np.zeros((128, L, QW, D), dtype=np.float32)
    for l in range(L):
        al = (0.4 * np.asarray(att[l], np.float64)).reshape(H * C).astype(np.float32)
        for q in range(QW):
            att_p[:, l, q, :] = al[None, :]

    bcols = np.zeros((128, 8), dtype=np.float32)
    for ch in range(2):
        for l in range(L):
            bcols[:, 2 * l + ch] = bconv[l][128 * ch:128 * (ch + 1)]
        bcols[:, 4 + ch] = Wpred[128 * ch:128 * (ch + 1), 0]

    shared = {
        "xaug": xaug.astype(NBF),
        "wp": wpb.astype(np.float32),
        "wf0": wf0.astype(NBF),
        "wr0f": wr0f.astype(NBF),
        "wl1": wl1_p.astype(NBF), "wr": wr_p.astype(NBF),
        "attrep": att_p.astype(NBF),
        "bcols": bcols,
        "i128": np.eye(128, dtype=np.float32),
        "i128b": np.eye(128, dtype=np.float32).astype(NBF),
        "epsc": np.full((128, 1), 1e-16, dtype=np.float32),
    }
    xtowns = []
    for c in range(NCORES):
        xt = np.ones((F_IN + 1, NPC), dtype=np.float32)
        xt[:F_IN, :] = xp[c * NPC + perms_old[c]].T
        xtowns.append(xt)
    return shared, xtowns


_CACHE = {}


def kernel(x, edge_index, batch, Wp, bp, Wl, Wr, att, bconv, Wpred, bpred,
           debug=False, _timing=None):
    x = np.asarray(x)
    edge_index = np.asarray(edge_index).astype(np.int64)
    batch = np.asarray(batch).astype(np.int64)

    t_blk, perms_old, perms_new, row_of, cores = _prep_edges(edge_index)
    shared, xtowns = _host_inputs(np.asarray(x), np.asarray(Wp), np.asarray(bp),
                                  np.asarray(Wl), np.asarray(Wr), np.asarray(att),
                                  np.asarray(bconv), np.asarray(Wpred),
                                  row_of, perms_old)

    key = (t_blk, bool(debug), os.environ.get("GAT_STAGE", "99"),
           os.environ.get("GAT_GBUFS", "3"), os.environ.get("GAT_EESPLIT", "0"))
    if key not in _CACHE:
        _CACHE[key] = _build(t_blk, debug=debug)
    nc = _CACHE[key]

    in_maps = []
    for c in range(NCORES):
        m = dict(shared)
        m["xtown"] = xtowns[c]
        m["xtownb"] = xtowns[c].astype(NBF)
        m["srcw"] = cores[c]["srcw"]
        m["ind"] = cores[c]["ind_pm"]
        m["indt"] = cores[c]["indt_pm"]
        in_maps.append(m)

    kw = {k: v for k, v in (_timing or {}).items() if k != "result"}
    res = run_bass_kernel_spmd(nc, in_maps, list(range(NCORES)), **kw)
    if _timing is not None:
        _timing["result"] = res

    pm = _pad_map()
    ycat = np.stack([res.results[c]["y_out"][0] for c in range(NCORES)])
    p = pm
    c_of = p // NPC
    r_new = np.concatenate([perms_new[c][None] for c in range(NCORES)])  # [8, NPC]
    y_real = ycat[c_of, r_new[c_of, p % NPC]]
    sums = np.bincount(batch, weights=y_real.astype(np.float64), minlength=G)
    cnt = np.bincount(batch, minlength=G).astype(np.float64)
    out = sums / np.maximum(cnt, 1.0) + float(np.asarray(bpred).reshape(-1)[0])
    if debug:
        return out.astype(np.float32)[:, None], res
    return out.astype(np.float32)[:, None]


# revision 71
# speedup vs baseline: 1.2010x; 1.1810x over previous
"""GATv2 (2-layer, 8-head) message-passing kernel for 8 Trainium2 NeuronCores.

Sharding: nodes (and incoming edges) are partitioned across 8 cores by
destination; weights replicated.  Per core, dst nodes are LPT-bin-packed into
40 blocks of 32 nodes (balancing edge counts so every block needs the same
t_blk=8 tiles of 128 edges).  Layer-0 source features xl0 are computed
replicated directly from x via the host-folded weight [Wp;bp]@Wl0
(contraction dim 8), written to DRAM (512B rows), and gathered per edge with
SWDGE.  Layer-1 xl is projected per chunk of 256 own nodes as layer-0 blocks
retire and a chunked AllGather distributes it, overlapping the collective
with compute.

SWDGE descriptor generation has a ~4.4us per-ucode-activation overhead but
batches all pending gather instructions into one activation, so gathers are
issued 8-at-a-time into one quad-sized buffer tile (whose WAR dependency
clears all at once); that drops the effective cost from ~9ns/edge to
~3ns/edge.  The sweep runs quads of 4 blocks in lockstep, tile by tile: the
4 subtiles of a tile-quad use PE row bands 32k (xr expand) and col bands 32k
(scatter), so the 4 matmuls of each step run concurrently on the PE array
and xr is read straight out of the resident projection tile (no replication
DMAs).  Per tile-quad: one batched ACT copy preloads the gathered xl into
PSUM, banded PE matmuls add the indicator-expanded xr (feat = xl[src] +
xr[dst]); ACT Abs, DVE mul by 0.4*att + 2-level per-head reduce into fp32 e
(the 0.6-linear source term rides 8 extra gathered columns; the dst linear
term cancels in the softmax), ACT exp, DVE alpha*xl, and banded PE
indicator-matmuls scatter [a | msg] into the 4 blocks' PSUM accumulator
strips.  Block epilogues accumulate node-major and one pair of [128,128]
transposes updates h per quad.  Layer-0 xr comes from x via the host-folded
[Wp;bp]@Wr0 (rank 8); layer-1 Wl/Wr projections run per 128-node chunk as
quads retire, with packed 528B-row AllGathers every 256 nodes re-pitched
locally to the 768B gather table.

bf16 everywhere on the edge path; accumulations (PSUM, h residual) fp32.
Self-contained: takes full (unsharded) inputs, returns the full output.
"""

import os
import numpy as np
from contextlib import ExitStack

import ml_dtypes
import concourse.bass as bass
import concourse.tile as tile
from concourse import bacc, mybir
from concourse.bass_utils import run_bass_kernel_spmd

# Problem constants (from the nn_GATv2Model spec)
N = 10000
E = 320000
F_IN = 7
D = 256
H = 8
C = 32
L = 2
G = 16
SLOPE = 0.2

NCORES = 8
NPC_REAL = 1250          # real nodes per core
NPC = 1280               # padded nodes per core (multiple of 256)
NP = NCORES * NPC        # padded total nodes
BLK = 32                 # dst nodes per block (indicator-matmul segment group)
NBLK = NPC // BLK        # blocks per core
PT = 128                 # edges per tile (partition dim)
CH = 5                   # AllGather chunks (NPC/256)
CHN = 256                # nodes per chunk
QW = 4                   # tiles per PSUM quad-group
DA = 8 + D               # scatter payload cols: 8 exp(e) + 256 msg
DG = D + 8               # gathered row: 256 xl + 8 folded source-linear
DPAD = 384               # xl_d row pitch in bf16 elems (768B, 256B multiple)

F32 = mybir.dt.float32
BF16 = mybir.dt.bfloat16
I16 = mybir.dt.int16
NBF = ml_dtypes.bfloat16


def _pad_map():
    n = np.arange(N)
    return (n // NPC_REAL) * NPC + (n % NPC_REAL)


def _prep_edges(edge_index: np.ndarray):
    """LPT-pack dst nodes into blocks, sort/shard/pad edges.

    Returns (t_blk, perms, cores) where perms[c] is old_of_new (new local
    id -> old padded-local id) and cores[c] has srcw/ind_pm/indt_pm."""
    pm = _pad_map()
    src_p = pm[edge_index[0]]
    dst_p = pm[edge_index[1]]
    owner = dst_p // NPC

    # ---- per-core LPT bin-packing of dst nodes into NBLK bins of 32 ----
    perms_new = []      # new_of_old per core
    perms_old = []      # old_of_new per core
    core_sel = []
    t_blk = 1
    for c in range(NCORES):
        sel = owner == c
        core_sel.append(sel)
        dl = dst_p[sel] - c * NPC
        deg = np.bincount(dl, minlength=NPC)
        order = np.argsort(-deg, kind="stable")
        binload = np.zeros(NBLK, np.int64)
        bincnt = np.zeros(NBLK, np.int64)
        assign = np.zeros(NPC, np.int64)
        for node in order:
            open_b = np.nonzero(bincnt < BLK)[0]
            b = open_b[np.argmin(binload[open_b])]
            assign[node] = b
            binload[b] += deg[node]
            bincnt[b] += 1
        new_of_old = np.zeros(NPC, np.int64)
        pos = np.zeros(NBLK, np.int64)
        for old in range(NPC):
            b = assign[old]
            new_of_old[old] = b * BLK + pos[b]
            pos[b] += 1
        perms_new.append(new_of_old)
        perms_old.append(np.argsort(new_of_old))
        t_blk = max(t_blk, int(np.ceil(binload.max() / PT)))
    t_blk = ((t_blk + QW - 1) // QW) * QW     # full q-groups

    # global chunk-major row map (256-node AllGather chunks): padded id ->
    # xl_d row
    row_of = np.zeros(NP, np.int64)
    for c in range(NCORES):
        r = perms_new[c]                      # old local -> new local
        j = r // CHN
        row_of[c * NPC:(c + 1) * NPC] = (j * NCORES + c) * CHN + (r % CHN)

    t8 = t_blk * 8
    ne_slots = NBLK * t_blk * PT
    cores = []
    for c in range(NCORES):
        sel = core_sel[c]
        es = row_of[src_p[sel]]                       # global xl_d rows
        ed = perms_new[c][dst_p[sel] - c * NPC]       # new local dst
        order = np.argsort(ed, kind="stable")
        es, ed = es[order], ed[order]
        blk = ed // BLK
        cnt = np.bincount(blk, minlength=NBLK)
        starts = np.zeros(NBLK, dtype=np.int64)
        starts[1:] = np.cumsum(cnt)[:-1]
        within = np.arange(len(es)) - starts[blk]
        # quad-interleaved slot order: slot of (block b = 4*qd+k4, within w)
        # lives at ((qd*t_blk + w//128)*4 + k4)*128 + w%128
        qdv, k4v = blk // 4, blk % 4
        slot = ((qdv * t_blk + within // PT) * 4 + k4v) * PT + within % PT

        src_slots = np.zeros(ne_slots, dtype=np.int64)
        dl32 = np.full(ne_slots, -1, dtype=np.int64)
        src_slots[slot] = es
        dl32[slot] = ed - blk * BLK

        # wrapped gather indices [128, ne_slots//16]: one 512-idx gather per
        # (qd, tt); slots are globally slot-major so the wrap is one reshape
        w16 = src_slots.reshape(-1, 16).T.astype(np.int16)   # [16, ne/16]
        srcw = np.tile(np.ascontiguousarray(w16), (8, 1))

        NQ = NBLK // 4
        ind = np.zeros((ne_slots, BLK), dtype=np.float32)
        valid = dl32 >= 0
        ind[np.nonzero(valid)[0], dl32[valid]] = 1.0
        ind = ind.reshape(NQ, t_blk, 4, PT, BLK)
        # scatter indicator per (qd, tt, k): ind_pm[128, qd, tt, k, 32]
        ind_pm = np.ascontiguousarray(
            ind.transpose(3, 0, 1, 2, 4)).astype(NBF)
        # transposed indicator: tile (qd, tt, k) at row band 32k:
        # indt_pm[32k:32k+32, qd, tt, :] = ind[qd, tt, k].T
        indt_pm = np.zeros((128, NQ, t_blk, PT), dtype=np.float32)
        for q in range(NQ):
            for tt in range(t_blk):
                for k4 in range(4):
                    indt_pm[32 * k4:32 * (k4 + 1), q, tt, :] = \
                        ind[q, tt, k4].T
        cores.append({"srcw": srcw, "ind_pm": ind_pm,
                      "indt_pm": indt_pm.astype(NBF)})
    return t_blk, perms_old, perms_new, row_of, cores


def _build(t_blk: int, debug: bool = False):
    """Build + compile the SPMD Bass program (identical on all cores)."""
    STAGE = int(os.environ.get("GAT_STAGE", "99"))
    GBUFS = int(os.environ.get("GAT_GBUFS", "3"))
    EESPLIT = os.environ.get("GAT_EESPLIT", "0") == "1"
    nt = NBLK * t_blk
    t8 = t_blk * 8

    nc = bacc.Bacc("TRN2", target_bir_lowering=False, debug=False,
                   num_devices=NCORES, num_swdge_queues=4)

    # ---- external inputs ----
    xaug = nc.dram_tensor("xaug", [F_IN + 1, NP], BF16, kind="ExternalInput").ap()
    xtown = nc.dram_tensor("xtown", [F_IN + 1, NPC], F32, kind="ExternalInput").ap()
    xtownb = nc.dram_tensor("xtownb", [F_IN + 1, NPC], BF16, kind="ExternalInput").ap()
    wr0f = nc.dram_tensor("wr0f", [F_IN + 1, D], BF16, kind="ExternalInput").ap()
    wp = nc.dram_tensor("wp", [F_IN + 1, D], F32, kind="ExternalInput").ap()
    wf0 = nc.dram_tensor("wf0", [F_IN + 1, DG], BF16, kind="ExternalInput").ap()
    wl1 = nc.dram_tensor("wl1", [128, 2, DG], BF16, kind="ExternalInput").ap()
    wr = nc.dram_tensor("wr", [128, 2, D], BF16, kind="ExternalInput").ap()
    attrep = nc.dram_tensor("attrep", [128, L, QW, D], BF16, kind="ExternalInput").ap()
    bcols = nc.dram_tensor("bcols", [128, 8], F32, kind="ExternalInput").ap()
    i128 = nc.dram_tensor("i128", [128, 128], F32, kind="ExternalInput").ap()
    epsc = nc.dram_tensor("epsc", [128, 1], F32, kind="ExternalInput").ap()
    NQ = NBLK // 4
    srcw_d = nc.dram_tensor("srcw", [128, NBLK * t8], I16, kind="ExternalInput").ap()
    ind_d = nc.dram_tensor("ind", [128, NQ, t_blk, 4, BLK], BF16,
                           kind="ExternalInput").ap()
    indt_d = nc.dram_tensor("indt", [128, NQ, t_blk, PT], BF16,
                            kind="ExternalInput").ap()

    # ---- internal DRAM ----
    warm_d = nc.dram_tensor("warm_d", [2, DPAD], BF16).ap()
    xl_d0 = nc.dram_tensor("xl_d0", [NP, DPAD], BF16).ap()
    stage = nc.dram_tensor("stage", [NPC, DG], BF16).ap()
    xl_d1p = nc.dram_tensor("xl_d1p", [NP, DG], BF16, addr_space="Shared").ap()
    xl_d1 = nc.dram_tensor("xl_d1", [NP, DPAD], BF16).ap()

    # ---- outputs ----
    y_out = nc.dram_tensor("y_out", [1, NPC], F32, kind="ExternalOutput").ap()
    if debug:
        hdump = nc.dram_tensor("hdump", [L, 2, 128, NPC], F32, kind="ExternalOutput").ap()

    with tile.TileContext(nc) as tc, ExitStack() as ctx:
        cpool = ctx.enter_context(tc.tile_pool(name="consts", bufs=1))
        drain = ctx.enter_context(tc.tile_pool(name="drain", bufs=4))
        gath = ctx.enter_context(tc.tile_pool(name="gath", bufs=GBUFS))
        indp = ctx.enter_context(tc.tile_pool(name="indp", bufs=4))
        indtp = ctx.enter_context(tc.tile_pool(name="indtp", bufs=4))
        upool = ctx.enter_context(tc.tile_pool(name="upool", bufs=4))
        appool = ctx.enter_context(tc.tile_pool(name="appool", bufs=4))
        epool = ctx.enter_context(tc.tile_pool(name="epool", bufs=4))
        astp = ctx.enter_context(tc.tile_pool(name="astp", bufs=4))
        smallp = ctx.enter_context(tc.tile_pool(name="smallp", bufs=3))
        outbp = ctx.enter_context(tc.tile_pool(name="outbp", bufs=3))

        # PSUM (8 banks): pfeat 4 (one [128,4,512] tile), pbig 3 (projection
        # psum + the 4-block col-tiled scatter accumulators), ptr 1
        pfeat = ctx.enter_context(tc.tile_pool(name="pfeat", bufs=1, space="PSUM"))
        pbig = ctx.enter_context(tc.tile_pool(name="pbig", bufs=3, space="PSUM"))
        ptrp = ctx.enter_context(tc.tile_pool(name="ptrp", bufs=1, space="PSUM"))

        def pmm_tile():
            ps = pbig.tile([128, 320], F32, tag="pb", name="pmm")
            return ps

        # ---- resident SBUF constants ----
        x_sb = cpool.tile([F_IN + 1, NP], BF16)
        nc.sync.dma_start(x_sb[:], xaug)
        xto_sb = cpool.tile([F_IN + 1, NPC], F32)
        nc.sync.dma_start(xto_sb[:], xtown)
        xtob_sb = cpool.tile([F_IN + 1, NPC], BF16)
        nc.sync.dma_start(xtob_sb[:], xtownb)
        wr0f_sb = cpool.tile([F_IN + 1, D], BF16)
        nc.sync.dma_start(wr0f_sb[:], wr0f)
        wp_sb = cpool.tile([F_IN + 1, D], F32)
        nc.sync.dma_start(wp_sb[:], wp)
        wf0_sb = cpool.tile([F_IN + 1, DG], BF16)
        nc.sync.dma_start(wf0_sb[:], wf0)
        wl1_sb = cpool.tile([128, 2, DG], BF16)
        nc.sync.dma_start(wl1_sb[:], wl1)
        wr_sb = cpool.tile([128, 2, D], BF16)
        nc.sync.dma_start(wr_sb[:], wr)
        att_sb = cpool.tile([128, L, QW, D], BF16)
        nc.sync.dma_start(att_sb[:], attrep)
        bc_sb = cpool.tile([128, 8], F32)
        nc.sync.dma_start(bc_sb[:], bcols)
        i128_sb = cpool.tile([128, 128], F32)
        nc.sync.dma_start(i128_sb[:], i128)
        eps_sb = cpool.tile([128, 1], F32)
        nc.sync.dma_start(eps_sb[:], epsc)
        srcw_sb = cpool.tile([128, NBLK * t8], I16)
        nc.sync.dma_start(srcw_sb[:], srcw_d)

        h_own = cpool.tile([128, 2, NPC], F32)     # h own slice, resident
        # xr projections, both layers, resident (band b%4 reads rows directly)
        xr_all = cpool.tile([128, L, NBLK // 4, D], BF16)

        nidx_reg = nc.gpsimd.to_reg(4 * PT)        # hoisted gather count reg

        # warmup: a throwaway gather so the SWDGE ucode activation cost
        # (~4.5us) is paid during the prologue, not the first sweep quad
        wix = cpool.tile([128, 8], I16)
        nc.vector.memset(wix[:], 0)
        warm = cpool.tile([128, 1, DPAD], BF16)
        nc.gpsimd.dma_gather(
            out_ap=warm[:], in_ap=warm_d,
            idxs_ap=wix[:], num_idxs=128, num_idxs_reg=128,
            elem_size=DPAD, queue_num=0)

        di = [0]

        def drain_ps(dst_ap, ps_ap):
            """PSUM -> SBUF copy alternating ACT/DVE to balance load."""
            if di[0] % 2 == 0:
                nc.scalar.activation(dst_ap, ps_ap,
                                     mybir.ActivationFunctionType.Copy)
            else:
                nc.vector.tensor_copy(dst_ap, ps_ap)
            di[0] += 1

        # ---- xl0 for ALL nodes straight from x (contraction dim 8),
        #      4 tiles per DRAM write ----
        for t4 in range(NP // 512 if STAGE >= 1 else 0):
            xw = drain.tile([128, 4, DG], BF16, tag="xw")
            for k in range(4):
                ps = pmm_tile()
                t = 4 * t4 + k
                nc.tensor.matmul(ps[:, 0:DG], x_sb[:, 128 * t:128 * (t + 1)],
                                 wf0_sb[:], start=True, stop=True)
                drain_ps(xw[:, k, :], ps[:, 0:DG])
            nc.sync.dma_start(
                xl_d0[512 * t4:512 * (t4 + 1), 0:DG]
                .rearrange("(f p) d -> p f d", p=128), xw[:])

        # ---- P0b: h0_own = [x|1] @ [Wp;bp] (feature-major, fp32) ----
        for k in range(4):
            for ch in range(2):
                ps = pmm_tile()
                nc.tensor.matmul(ps[:, 0:320], wp_sb[:, 128 * ch:128 * (ch + 1)],
                                 xto_sb[:, 320 * k:320 * (k + 1)],
                                 start=True, stop=True)
                drain_ps(h_own[:, ch, 320 * k:320 * (k + 1)], ps[:, 0:320])

        # ---- layers ----
        for l in range(L if STAGE >= 2 else 0):
            if l == 0:
                # xr0 = [x|1] @ (Wpb@Wr0): rank-8 bf16 matmul straight from
                # the core's own x -- no dependence on h0
                for jo in range(10 if STAGE >= 3 else 0):
                    ps = pmm_tile()
                    nc.tensor.matmul(ps[:, 0:D],
                                     xtob_sb[:, 128 * jo:128 * (jo + 1)],
                                     wr0f_sb[:], start=True, stop=True)
                    drain_ps(xr_all[:, l, jo, :], ps[:, 0:D])

            # edge sweep: quads of 4 blocks in lockstep, tile by tile; the
            # 4 subtiles of a tile-quad use PE row bands 32k (expand) and
            # col bands 32k (scatter), so LDWEIGHTS overlaps MATMUL
            nq_run = NQ if STAGE >= 4 else 0
            for qd in range(nq_run):
                # one buffer tile per quad: all 8 gathers' WAR deps clear at
                # once when the previous quad retires, so they arrive at the
                # GpSimd queue together and one ucode activation batches them
                xq = gath.tile([128, t_blk, 4, DPAD], BF16, tag="g")
                for tt in range(t_blk):
                    gi = qd * t_blk + tt
                    nc.gpsimd.dma_gather(
                        out_ap=xq[:, tt, :, :],
                        in_ap=xl_d0 if l == 0 else xl_d1,
                        idxs_ap=srcw_sb[:, gi * 32:(gi + 1) * 32],
                        num_idxs=4 * PT,
                        num_idxs_reg=nidx_reg,
                        elem_size=DPAD,
                        queue_num=gi % 4,
                    )
                xs = [xq[:, tt, :, :] for tt in range(t_blk)]
                ind_sb = indp.tile([128, t_blk, 4, BLK], BF16)
                nc.sync.dma_start(ind_sb[:], ind_d[:, qd, :, :, :])
                indt_sb = indtp.tile([128, t_blk, PT], BF16)
                nc.sync.dma_start(indt_sb[:], indt_d[:, qd, :, :])

                pb4 = pbig.tile([128, 320], F32, tag="pb")

                for tt in range(t_blk):
                    xsrc = xs[tt]
                    pf = pfeat.tile([128, QW, 512], F32, tag="pf")
                    # batched preload of the whole tile-quad into PSUM
                    nc.scalar.activation(
                        pf[:, 0:4, 0:D], xsrc[:, :, 0:D],
                        mybir.ActivationFunctionType.Copy)
                    for k in range(4):
                        nc.tensor.matmul(pf[:, k, 0:D],
                                         indt_sb[32 * k:32 * (k + 1), tt, :],
                                         xr_all[32 * k:32 * (k + 1), l, qd, :],
                                         start=False, stop=True,
                                         skip_group_check=True,
                                         tile_position=(32 * k, 0))
                    u = upool.tile([128, QW, D], BF16)
                    nc.scalar.activation(u[:], pf[:, 0:4, 0:D],
                                         mybir.ActivationFunctionType.Abs)
                    ap_t = appool.tile([128, QW, D], BF16)
                    nc.vector.tensor_mul(ap_t[:], u[:], att_sb[:, l, :, :])
                    # per-head reduce as a 2-level 2x-mode tree + short reduce
                    apv = ap_t[:].rearrange("p a (h c) -> p a h c", h=H)
                    t1 = epool.tile([128, QW, H, 16], BF16, tag="t1")
                    nc.vector.tensor_add(t1[:], apv[:, :, :, 0:16],
                                         apv[:, :, :, 16:32])
                    e_sb = epool.tile([128, QW, 8], F32, tag="e")
                    with nc.allow_low_precision("bf16 partial sums"):
                        nc.vector.tensor_reduce(
                            e_sb[:], t1[:],
                            axis=mybir.AxisListType.X, op=mybir.AluOpType.add)
                    nc.vector.tensor_add(e_sb[:], e_sb[:], xsrc[:, :, D:DG])
                    ast = astp.tile([128, QW, DA], BF16)
                    nc.scalar.activation(ast[:, :, 0:8], e_sb[:],
                                         mybir.ActivationFunctionType.Exp)
                    if EESPLIT:
                        # expand exp per-head on ACT (which has slack with
                        # the preload on PE) so the DVE mul runs in 2x mode
                        eexp = epool.tile([128, QW, D], BF16, tag="ee")
                        nc.scalar.activation(
                            eexp[:].rearrange("p a (h c) -> p a h c", h=H),
                            e_sb[:].unsqueeze(-1).broadcast_to([128, QW, H, C]),
                            mybir.ActivationFunctionType.Exp)
                        nc.vector.tensor_mul(ast[:, :, 8:], xsrc[:, :, 0:D],
                                             eexp[:])
                    else:
                        nc.vector.tensor_mul(
                            ast[:, :, 8:].rearrange("p a (h c) -> p a h c", h=H),
                            xsrc[:, :, 0:D].rearrange("p a (h c) -> p a h c", h=H),
                            ast[:, :, 0:8].unsqueeze(-1).broadcast_to(
                                [128, 4, H, C]))
                    for k in range(4):
                        nc.tensor.matmul(pb4[32 * k:32 * (k + 1), 0:DA],
                                         ind_sb[:, tt, k, :], ast[:, k, :],
                                         start=(tt == 0), stop=(tt == t_blk - 1),
                                         skip_group_check=True,
                                         tile_position=(0, 32 * k))

                # 4 block epilogues into one node-major quad tile, then a
                # single pair of [128,128] transposes + h update
                hq = outbp.tile([128, D], F32)
                for k in range(4):
                    cj = 32 * k
                    pb = pb4[cj:cj + 32, 0:DA]
                    dsb = smallp.tile([128, 8], F32, tag="dsb")
                    nc.scalar.activation(dsb[cj:cj + 32, :], pb[:, 0:8],
                                         mybir.ActivationFunctionType.Identity,
                                         bias=eps_sb[cj:cj + 32, :])
                    dinv = smallp.tile([128, 8], F32, tag="dinv")
                    nc.vector.reciprocal(dinv[cj:cj + 32, :], dsb[cj:cj + 32, :])
                    nc.vector.tensor_mul(
                        hq[cj:cj + 32, :].rearrange("p (h c) -> p h c", h=H),
                        pb[:, 8:].rearrange("p (h c) -> p h c", h=H),
                        dinv[cj:cj + 32, :].unsqueeze(-1).broadcast_to([32, H, C]))
                pt = ptrp.tile([128, 2, 128], F32)
                for ch in range(2):
                    nc.tensor.transpose(pt[:, ch, :],
                                        hq[:, 128 * ch:128 * (ch + 1)],
                                        i128_sb[:])
                for ch in range(2):
                    nc.vector.tensor_add(
                        h_own[:, ch, 128 * qd:128 * (qd + 1)],
                        h_own[:, ch, 128 * qd:128 * (qd + 1)],
                        pt[:, ch, :])

                # after every quad in layer 0: bconv the finished 128 own
                # nodes, project through Wl1 (stage + AllGather chunk) and
                # Wr[1] (xr for layer 1)
                n0 = 128 * qd
                if l == 0 and STAGE >= 5:
                    for ch in range(2):
                        nc.vector.tensor_scalar_add(
                            h_own[:, ch, n0:n0 + 128],
                            h_own[:, ch, n0:n0 + 128],
                            bc_sb[:, 2 * l + ch:2 * l + ch + 1])
                    hb = drain.tile([128, 2, 128], BF16, tag="hb")
                    drain_ps(hb[:], h_own[:, :, n0:n0 + 128])
                    ps = pmm_tile()
                    for ch in range(2):
                        nc.tensor.matmul(ps[:, 0:DG], hb[:, ch, :],
                                         wl1_sb[:, ch, :],
                                         start=(ch == 0), stop=(ch == 1))
                    stg = drain.tile([128, DG], BF16, tag="stg")
                    drain_ps(stg[:], ps[:, 0:DG])
                    nc.sync.dma_start(stage[n0:n0 + 128, :], stg[:])
                    ps2 = pmm_tile()
                    for ch in range(2):
                        nc.tensor.matmul(ps2[:, 0:D], hb[:, ch, :],
                                         wr_sb[:, ch, :],
                                         start=(ch == 0), stop=(ch == 1))
                    drain_ps(xr_all[:, 1, qd, :], ps2[:, 0:D])
                    if qd % 2 == 1:
                        # AllGather packed 528B rows (31% fewer bytes than
                        # the padded gather pitch), then re-pitch locally
                        j = qd // 2
                        g0 = NCORES * CHN * j
                        nc.gpsimd.collective_compute(
                            "AllGather", mybir.AluOpType.bypass,
                            replica_groups=[list(range(NCORES))],
                            ins=[stage[CHN * j:CHN * (j + 1), :]],
                            outs=[xl_d1p[g0:g0 + NCORES * CHN, :]])
                        nc.sync.dma_start(
                            xl_d1[g0:g0 + NCORES * CHN, 0:DG],
                            xl_d1p[g0:g0 + NCORES * CHN, :])
                if l == 1 and STAGE >= 5:
                    # incremental final: bconv + y for the finished chunk
                    for ch in range(2):
                        nc.vector.tensor_scalar_add(
                            h_own[:, ch, n0:n0 + 128],
                            h_own[:, ch, n0:n0 + 128],
                            bc_sb[:, 2 * l + ch:2 * l + ch + 1])
                    ps = pmm_tile()
                    for ch in range(2):
                        nc.tensor.matmul(ps[0:1, 0:128], bc_sb[:, 4 + ch:5 + ch],
                                         h_own[:, ch, n0:n0 + 128],
                                         start=(ch == 0), stop=(ch == 1))
                    ysb = drain.tile([1, 128], F32, tag="ysb")
                    nc.scalar.activation(ysb[0:1, :], ps[0:1, 0:128],
                                         mybir.ActivationFunctionType.Copy)
                    nc.sync.dma_start(y_out[0:1, n0:n0 + 128], ysb[0:1, :])

            if STAGE < 5:
                # bconv for the whole slice at once
                for ch in range(2):
                    nc.vector.tensor_scalar_add(
                        h_own[:, ch, :], h_own[:, ch, :],
                        bc_sb[:, 2 * l + ch:2 * l + ch + 1])

            if debug:
                nc.sync.dma_start(hdump[l].rearrange("c p n -> p c n"), h_own[:])

        if STAGE < 5:
            # ---- final: y = h_own @ Wpred (non-incremental fallback) ----
            for k in range(4):
                w = 320
                ps = pmm_tile()
                for ch in range(2):
                    nc.tensor.matmul(ps[0:1, 0:w], bc_sb[:, 4 + ch:5 + ch],
                                     h_own[:, ch, 320 * k:320 * k + w],
                                     start=(ch == 0), stop=(ch == 1))
                ysb = drain.tile([1, 320], F32, tag="ysb")
                nc.scalar.activation(ysb[0:1, 0:w], ps[0:1, 0:w],
                                     mybir.ActivationFunctionType.Copy)
                nc.sync.dma_start(y_out[0:1, 320 * k:320 * k + w], ysb[0:1, 0:w])

    nc.compile()
    return nc


def _host_inputs(x, Wp, bp, Wl, Wr, att, bconv, Wpred, row_of, perms_old):
    pm = _pad_map()
    xp = np.zeros((NP, F_IN), dtype=np.float32)
    xp[pm] = np.asarray(x, dtype=np.float32)

    # augmented per-layer projections [Wl | Wl@A], A = 0.6*att blockdiag
    wla = []
    for l in range(L):
        A = np.zeros((D, H), dtype=np.float64)
        for hh in range(H):
            A[hh * C:(hh + 1) * C, hh] = 0.6 * att[l][hh]
        wla.append(np.concatenate([Wl[l], Wl[l] @ A], axis=1))  # [256, 264]

    # layer-0: fold through Wp (x is only 7-dim + ones column for biases)
    wpb = np.concatenate([Wp, bp[None, :]], axis=0)             # [8, 256]
    wf0 = wpb @ wla[0]                                          # [8, 264]
    wr0f = wpb @ Wr[0]                                          # [8, 256]

    xaug = np.ones((F_IN + 1, NP), dtype=np.float32)
    xaug[:F_IN, :] = 0.0
    xaug[:F_IN, row_of] = xp.T                                  # chunk-major cols

    wl1_p = np.zeros((128, 2, DG), dtype=np.float32)
    wr_p = np.zeros((128, 2, D), dtype=np.float32)
    for ch in range(2):
        wl1_p[:, ch, :] = wla[1][128 * ch:128 * (ch + 1), :]
        wr_p[:, ch, :] = Wr[1][128 * ch:128 * (ch + 1), :]
    att_p = np.zeros((128, L, QW, D), dtype=np.float32)
    for l in range(L):
        al = (0.4 * np.asarray(att[l], np.float64)).reshape(H * C).astype(np.float32)
        for q in range(QW):
            att_p[:, l, q, :] = al[None, :]

    bcols = np.zeros((128, 8), dtype=np.float32)
    for ch in range(2):
        for l in range(L):
            bcols[:, 2 * l + ch] = bconv[l][128 * ch:128 * (ch + 1)]
        bcols[:, 4 + ch] = Wpred[128 * ch:128 * (ch + 1), 0]

    shared = {
        "xaug": xaug.astype(NBF),
        "wp": wpb.astype(np.float32),
        "wf0": wf0.astype(NBF),
        "wr0f": wr0f.astype(NBF),
        "wl1": wl1_p.astype(NBF), "wr": wr_p.astype(NBF),
        "attrep": att_p.astype(NBF),
        "bcols": bcols,
        "i128": np.eye(128, dtype=np.float32),
        "epsc": np.full((128, 1), 1e-16, dtype=np.float32),
    }
    xtowns = []
    for c in range(NCORES):
        xt = np.ones((F_IN + 1, NPC), dtype=np.float32)
        xt[:F_IN, :] = xp[c * NPC + perms_old[c]].T
        xtowns.append(xt)
    return shared, xtowns


_CACHE = {}


def kernel(x, edge_index, batch, Wp, bp, Wl, Wr, att, bconv, Wpred, bpred,
           debug=False, _timing=None):
    x = np.asarray(x)
    edge_index = np.asarray(edge_index).astype(np.int64)
    batch = np.asarray(batch).astype(np.int64)

    t_blk, perms_old, perms_new, row_of, cores = _prep_edges(edge_index)
    shared, xtowns = _host_inputs(np.asarray(x), np.asarray(Wp), np.asarray(bp),
                                  np.asarray(Wl), np.asarray(Wr), np.asarray(att),
                                  np.asarray(bconv), np.asarray(Wpred),
                                  row_of, perms_old)

    key = (t_blk, bool(debug), os.environ.get("GAT_STAGE", "99"),
           os.environ.get("GAT_GBUFS", "3"), os.environ.get("GAT_EESPLIT", "0"))
    if key not in _CACHE:
        _CACHE[key] = _build(t_blk, debug=debug)
    nc = _CACHE[key]

    in_maps = []
    for c in range(NCORES):
        m = dict(shared)
        m["xtown"] = xtowns[c]
        m["xtownb"] = xtowns[c].astype(NBF)
        m["srcw"] = cores[c]["srcw"]
        m["ind"] = cores[c]["ind_pm"]
        m["indt"] = cores[c]["indt_pm"]
        in_maps.append(m)

    kw = {k: v for k, v in (_timing or {}).items() if k != "result"}
    res = run_bass_kernel_spmd(nc, in_maps, list(range(NCORES)), **kw)
    if _timing is not None:
        _timing["result"] = res

    pm = _pad_map()
    ycat = np.stack([res.results[c]["y_out"][0] for c in range(NCORES)])
    p = pm
    c_of = p // NPC
    r_new = np.concatenate([perms_new[c][None] for c in range(NCORES)])  # [8, NPC]
    y_real = ycat[c_of, r_new[c_of, p % NPC]]
    sums = np.bincount(batch, weights=y_real.astype(np.float64), minlength=G)
    cnt = np.bincount(batch, minlength=G).astype(np.float64)
    out = sums / np.maximum(cnt, 1.0) + float(np.asarray(bpred).reshape(-1)[0])
    if debug:
        return out.astype(np.float32)[:, None], res
    return out.astype(np.float32)[:, None]
